# revision 1
# baseline (speedup 1.0000x reference)
"""Self-contained Trainium2 kernel for the Devign GatedGraphConv problem.

kernel(**inputs) -> [1,2] softmax, computed on 8 NeuronCores via Bass SPMD.
"""


import math
import contextlib
import numpy as np

def _nullcm():
    return contextlib.nullcontext()

# ---------------------------------------------------------------- config ----

class Cfg:
    def __init__(self, n_nodes=100000, n_edges=400000, nsh=12544,
                 in_dim=100, d=200, g=600, num_steps=4, nc=8,
                 bin_sz=96, gbins=6, nchunk=256):
        self.N = n_nodes
        self.E = n_edges
        self.NSH = nsh                      # node slots per core (mult of 128)
        self.NC = nc
        self.NPAD = nsh * nc
        self.IN_DIM = in_dim
        self.D = d                          # out feature dim (200)
        self.DP = 256                       # padded row width for gather
        self.G = g                          # 3*d gate width
        self.STEPS = num_steps
        self.NBUCKET = 4
        self.BROWS = self.NPAD // 4         # gather bucket rows (int16-safe)
        self.BIN = bin_sz                   # dst window per segsum bin
        self.NBINS = math.ceil(nsh / bin_sz)
        self.GBINS = gbins                  # bins per processing group
        self.NCHUNK = min(nchunk, nsh)      # GRU node-chunk width
        self.PACK = 5                       # segsum bins per psum bank
        assert nsh % 128 == 0
        assert self.BROWS <= 32768
        assert self.BROWS % 1 == 0


# ---------------------------------------------------------- host planning ----

class Plan:
    """Static (data-dependent but core-uniform) edge/chunk structure."""
    pass


def build_plan(cfg: Cfg, edge_index: np.ndarray):
    src = edge_index[0].astype(np.int64)
    dst = edge_index[1].astype(np.int64)
    ecore = dst // cfg.NSH
    dstloc = dst % cfg.NSH
    ebin = dstloc // cfg.BIN
    ebuck = src // cfg.BROWS

    NB, NK, NCC = cfg.NBINS, cfg.NBUCKET, cfg.NC
    # counts[c, b, k]
    counts = np.zeros((NCC, NB, NK), np.int64)
    np.add.at(counts, (ecore, ebin, ebuck), 1)
    # chunks per (bin, bucket): max over cores
    kchunks = np.ceil(counts.max(axis=0) / 128).astype(np.int64)  # [NB, NK]
    # ensure every bin has at least one chunk so its psum accum is defined
    empty = kchunks.sum(axis=1) == 0
    kchunks[empty, 0] = 1

    # bucket-major slot layout: bucket k holds, bin-by-bin, kchunks[b,k]*128
    # slots; S[k] = bucket k slot base, off[b,k] = within-bucket offset
    Lk = (kchunks.sum(axis=0) * 128).astype(np.int64)       # slots per bucket
    Sk = np.zeros(NK + 1, np.int64)
    np.cumsum(Lk, out=Sk[1:])
    off = np.zeros((NB, NK), np.int64)
    for k in range(NK):
        off[1:, k] = np.cumsum(kchunks[:-1, k] * 128)
    TOT = int(Sk[-1])

    # bin-major chunk table: (bin, bucket, slot_start)
    chunk_tab = []
    chunks_of_bin = [[] for _ in range(NB)]
    for b in range(NB):
        for k in range(NK):
            for j in range(int(kchunks[b, k])):
                cb = len(chunk_tab)
                sl = int(Sk[k] + off[b, k] + j * 128)
                chunk_tab.append((b, k, sl))
                chunks_of_bin[b].append(cb)
    TOTCH = len(chunk_tab)

    # slot -> (bin-major chunk id) mapping for dstoff placement
    # per-core arrays
    idx_all = np.zeros((NCC, TOT), np.int16)
    dstoff_all = np.full((NCC, 128, TOTCH), -1.0, np.float32)

    # chunk slot bases per (b, k) in slot space and in chunk-id space
    cb_base = np.zeros((NB, NK), np.int64)
    for b in range(NB):
        run = 0
        for k in range(NK):
            cb_base[b, k] = chunks_of_bin[b][run] if kchunks[b, k] > 0 else -1
            run += int(kchunks[b, k])

    order = np.lexsort((dstloc, ebuck, ebin, ecore))
    so, do_, co, bo, ko = (src[order], dstloc[order], ecore[order],
                           ebin[order], ebuck[order])
    # group boundaries over (core, bin, bucket)
    key = (co * NB + bo) * NK + ko
    bounds = np.flatnonzero(np.r_[True, key[1:] != key[:-1], True])
    for i0, i1 in zip(bounds[:-1], bounds[1:]):
        c, b, k = int(co[i0]), int(bo[i0]), int(ko[i0])
        n = i1 - i0
        base_slot = int(Sk[k] + off[b, k])
        idx_all[c, base_slot:base_slot + n] = (so[i0:i1] - k * cfg.BROWS
                                               ).astype(np.int16)
        # dstoff[c, e%128, cb_base + e//128]
        e = np.arange(n)
        cbs = cb_base[b, k] + e // 128
        dstoff_all[c, e % 128, cbs] = (do_[i0:i1] - b * cfg.BIN).astype(
            np.float32)

    # idx arrays -> [128, TOT//16] int16 wrapped layout
    assert TOT % 128 == 0
    idx_wrapped = np.zeros((NCC, 128, TOT // 16), np.int16)
    for c in range(NCC):
        w = idx_all[c].reshape(TOT // 16, 16).T          # [16, TOT//16]
        idx_wrapped[c] = np.tile(w, (8, 1))

    # processing groups of bins
    groups = []
    for g0 in range(0, NB, cfg.GBINS):
        g1 = min(g0 + cfg.GBINS, NB)
        cb0 = chunks_of_bin[g0][0]
        cb1 = chunks_of_bin[g1 - 1][-1] + 1
        # per bucket: slot range and chunk count for this group
        brange = []
        for k in range(NK):
            s0 = int(Sk[k] + off[g0, k])
            s1 = int(Sk[k] + (off[g1, k] if g1 < NB else
                              off[g1 - 1, k] + kchunks[g1 - 1, k] * 128))
            brange.append((s0, s1))
        groups.append(dict(bins=(g0, g1), cbs=(cb0, cb1), brange=brange))

    p = Plan()
    p.kchunks, p.Sk, p.off = kchunks, Sk, off
    p.TOT, p.TOTCH = TOT, TOTCH
    p.chunk_tab, p.chunks_of_bin = chunk_tab, chunks_of_bin
    p.groups = groups
    p.idx_wrapped = idx_wrapped
    p.dstoff = dstoff_all
    return p


def host_inputs(cfg: Cfg, plan, inputs):
    """Build per-core in_maps (numpy) for the device program."""
    import ml_dtypes
    bf16 = ml_dtypes.bfloat16
    x = np.asarray(inputs["x"], np.float32)
    W = np.asarray(inputs["W"], np.float32)
    w_ih = np.asarray(inputs["w_ih"], np.float32)
    w_hh = np.asarray(inputs["w_hh"], np.float32)
    b_ih = np.asarray(inputs["b_ih"], np.float32)
    b_hh = np.asarray(inputs["b_hh"], np.float32)

    D, G, S = cfg.D, cfg.G, cfg.STEPS
    # W moving operand [S, 2, 128, D]
    W_rhs = np.zeros((S, 2, 128, D), np.float32)
    W_rhs[:, 0] = W[:, 0:128, :]
    W_rhs[:, 1, 0:D - 128] = W[:, 128:D, :]
    # gate weights stationary [2, 128, G] with bias row at grp1 row D-128
    def lhsT(wmat, bias):
        out = np.zeros((2, 128, G), np.float32)
        wT = wmat.T                      # [D, G]
        out[0] = wT[0:128]
        out[1, 0:D - 128] = wT[128:D]
        out[1, 96] = bias
        return out
    wih_l = lhsT(w_ih, b_ih)
    whh_l = lhsT(w_hh, b_hh)

    shared = {
        "w_rhs": W_rhs.astype(bf16),
        "wih": wih_l.astype(bf16),
        "whh": whh_l.astype(bf16),
    }

    in_maps = []
    for c in range(cfg.NC):
        lo = c * cfg.NSH
        hi = min((c + 1) * cfg.NSH, cfg.N)
        nreal = max(0, hi - lo)
        xT = np.zeros((128, cfg.NSH), np.float32)
        if nreal > 0:
            xT[0:cfg.IN_DIM, 0:nreal] = x[lo:hi].T
        mask = np.zeros((128, cfg.NSH), np.float32)
        mask[:, 0:nreal] = 1.0
        m = dict(shared)
        m["xT"] = xT.astype(bf16)
        m["mask"] = mask.astype(bf16)
        m["idx"] = plan.idx_wrapped[c]
        st = (plan.dstoff[c][:, :, None] ==
              np.arange(cfg.BIN, dtype=np.float32)[None, None, :])
        m["st"] = st.astype(bf16)
        in_maps.append(m)
    return in_maps


# ------------------------------------------------------------ device build ----

def build_program(cfg: Cfg, plan, timing_mode=False, skip=(), standin8=False):
    import concourse.bass as bass
    import concourse.bacc as bacc
    import concourse.tile as tile
    import concourse.mybir as mybir
    dt = mybir.dt
    AF = mybir.ActivationFunctionType
    ALU = mybir.AluOpType

    NSH, D, G, DP = cfg.NSH, cfg.D, cfg.G, cfg.DP
    DH = D - 128            # hi-group feature count (72)
    ONE = 96                # ones row (32-aligned partition) for bias folding
    KH = ONE + 1            # hi-group contraction rows incl ones row
    NB, NK = cfg.NBINS, cfg.NBUCKET
    BIN = cfg.BIN
    NCH = cfg.NCHUNK
    PACK = cfg.PACK         # bins per psum bank in segsum
    GRPM = 8                # m chunks per HBM write

    nc = bacc.Bacc("TRN2", target_bir_lowering=False, debug=False,
                   num_devices=1 if timing_mode else cfg.NC)

    xT_in = nc.dram_tensor("xT", [128, NSH], dt.bfloat16, kind="ExternalInput")
    mask_in = nc.dram_tensor("mask", [128, NSH], dt.bfloat16,
                             kind="ExternalInput")
    idx_in = nc.dram_tensor("idx", [128, plan.TOT // 16], dt.int16,
                            kind="ExternalInput")
    st_in = nc.dram_tensor("st", [128, plan.TOTCH, BIN], dt.bfloat16,
                           kind="ExternalInput")
    wrhs_in = nc.dram_tensor("w_rhs", [cfg.STEPS, 2, 128, D], dt.bfloat16,
                             kind="ExternalInput")
    wih_in = nc.dram_tensor("wih", [2, 128, G], dt.bfloat16,
                            kind="ExternalInput")
    whh_in = nc.dram_tensor("whh", [2, 128, G], dt.bfloat16,
                            kind="ExternalInput")
    y_out = nc.dram_tensor("y", [128, 2], dt.float32, kind="ExternalOutput")

    import contextlib as _ctxlib
    with tile.TileContext(nc) as tc:
        with tc.tile_pool(name="persist", bufs=1) as pp, \
             tc.tile_pool(name="dram", bufs=1, space="DRAM") as dram:
            h_lo = pp.tile([128, NSH], dt.bfloat16, tag="h_lo")
            h_hi = pp.tile([128, NSH], dt.bfloat16, tag="h_hi")
            agg_lo = pp.tile([128, NB * BIN], dt.bfloat16, tag="agg_lo")
            agg_hi = pp.tile([128, NB * BIN], dt.bfloat16, tag="agg_hi")
            idx_s = pp.tile([128, plan.TOT // 16], dt.int16, tag="idx")
            wrhs_s = pp.tile([128, cfg.STEPS, 2, D], dt.bfloat16, tag="wrhs")
            wih_s = pp.tile([128, 2, G], dt.bfloat16, tag="wih")
            whh_s = pp.tile([128, 2, G], dt.bfloat16, tag="whh")

            m_mines = [dram.tile([NSH, DP], dt.bfloat16, tag=f"m_mine{s}",
                                 name=f"m_mine{s}")
                       for s in range(cfg.STEPS)]
            # Shared DRAM wants a single writer: one all-gather dst per step
            m_fulls = [dram.tile([cfg.NC, NSH, DP], dt.bfloat16,
                                 addr_space="Local" if (timing_mode or
                                                        standin8)
                                 else "Shared", tag=f"m_full{s}",
                                 name=f"m_full{s}")
                       for s in range(cfg.STEPS)]

            # loads
            nc.sync.dma_start(h_lo[:], xT_in.ap())
            nc.sync.dma_start(idx_s[:], idx_in.ap())
            nc.sync.dma_start(wrhs_s[:],
                              wrhs_in.ap().rearrange("s g p m -> p s g m"))
            nc.sync.dma_start(wih_s[:],
                              wih_in.ap().rearrange("g p m -> p g m"))
            nc.sync.dma_start(whh_s[:],
                              whh_in.ap().rearrange("g p m -> p g m"))
            nc.vector.memset(h_hi[:], 0.0)
            nc.vector.memset(agg_hi[:], 0.0)
            nc.vector.memset(h_hi[ONE:ONE + 1, :], 1.0)
            nc.vector.memset(agg_hi[ONE:ONE + 1, :], 1.0)

            NCHK = NSH // 128            # 128-node chunks for phase A
            phase_stack = _ctxlib.ExitStack()
            pmA = phase_stack.enter_context(
                tc.tile_pool(name="pmA", bufs=2, space="PSUM"))
            sbA = phase_stack.enter_context(tc.tile_pool(name="sbA", bufs=2))
            msgB = phase_stack.enter_context(tc.tile_pool(name="msgB", bufs=2))
            stB = phase_stack.enter_context(tc.tile_pool(name="stB", bufs=2))
            psB = phase_stack.enter_context(
                tc.tile_pool(name="psB", bufs=1, space="PSUM"))
            psC = phase_stack.enter_context(
                tc.tile_pool(name="psC", bufs=1, space="PSUM"))
            sbC = phase_stack.enter_context(tc.tile_pool(name="sbC", bufs=2))
            for step in range(cfg.STEPS):
                m_mine = m_mines[step]
                # ------- phase A: m = h @ W[step] (node-major, batched) ----
                for c0 in range(0, NCHK, GRPM) if "m" not in skip else []:
                    c1 = min(c0 + GRPM, NCHK)
                    msb = sbA.tile([128, GRPM, D], dt.bfloat16, tag="msb")
                    for cp in range(c0, c1, 2):
                        npair = min(2, c1 - cp)
                        pm = pmA.tile([128, 2, D], dt.float32, tag="pm")
                        for j in range(npair):
                            sl = slice((cp + j) * 128, (cp + j + 1) * 128)
                            nc.tensor.matmul(pm[:, j, :], h_lo[:, sl],
                                             wrhs_s[:, step, 0, :],
                                             start=True, stop=False)
                            nc.tensor.matmul(pm[:, j, :], h_hi[0:DH, sl],
                                             wrhs_s[0:DH, step, 1, :],
                                             start=False, stop=True)
                        nc.scalar.activation(
                            msb[:, cp - c0:cp - c0 + npair, :],
                            pm[:, 0:npair, :], AF.Copy)
                    ngrp = c1 - c0
                    nc.sync.dma_start(
                        m_mine[c0 * 128:c1 * 128, 0:D].rearrange(
                            "(c p) d -> p c d", p=128),
                        msb[:, 0:ngrp, :])

                # ---------------- all-gather m ----------------------------
                m_full = m_fulls[step]
                m_flat = m_full.rearrange("c n d -> (c n) d")
                if timing_mode or standin8:
                    # stand-in for the collective: same local write volume
                    for r in range(cfg.NC) if "ag" not in skip else []:
                        nc.sync.dma_start(m_full[r], m_mine[:])
                elif True:
                    nc.gpsimd.collective_compute(
                        "AllGather", ALU.bypass,
                        replica_groups=[list(range(cfg.NC))],
                        ins=[m_mine.opt()], outs=[m_full.opt()])

                # ---------------- phase B: gather + segment-sum ------------
                for grp in plan.groups:
                    g0, g1 = grp["bins"]
                    cb0, cb1 = grp["cbs"]
                    nch_g = cb1 - cb0
                    st_t = stB.tile([128, nch_g, BIN], dt.bfloat16, tag="st")
                    nc.sync.dma_start(st_t[:], st_in.ap()[:, cb0:cb1, :])

                    nch_tot = sum((grp["brange"][k][1] - grp["brange"][k][0])
                                  for k in range(NK)) // 128
                    mt = msgB.tile([128, max(nch_tot, 1), DP], dt.bfloat16,
                                   tag="msg")
                    msg_t = {}
                    moff = 0
                    for k in range(NK) if "gather" not in skip else []:
                        s0, s1 = grp["brange"][k]
                        nsl = s1 - s0
                        if nsl == 0:
                            continue
                        nc.gpsimd.dma_gather(
                            out_ap=mt[:, moff:moff + nsl // 128, :],
                            in_ap=m_flat[k * cfg.BROWS:(k + 1) * cfg.BROWS, :],
                            idxs_ap=idx_s[:, s0 // 16:s1 // 16],
                            num_idxs=nsl,
                            num_idxs_reg=nsl,
                            elem_size=DP,
                            single_packet=False)
                        msg_t[k] = (s0, moff)
                        moff += nsl // 128

                    for b0 in range(g0, g1, PACK) if "seg" not in skip else []:
                        b1 = min(b0 + PACK, g1)
                        npk = b1 - b0
                        plo = psB.tile([128, PACK, BIN], dt.float32, tag="plo")
                        phi = psB.tile([128, PACK, BIN], dt.float32, tag="phi")
                        for b in range(b0, b1):
                            j = b - b0
                            cbs = plan.chunks_of_bin[b]
                            for ci, cb in enumerate(cbs):
                                _, k, sl0 = plan.chunk_tab[cb]
                                s0, moff_k = msg_t[k]
                                lsl = moff_k + (sl0 - s0) // 128
                                first, last = ci == 0, ci == len(cbs) - 1
                                nc.tensor.matmul(
                                    plo[:, j, :], mt[:, lsl, 0:128],
                                    st_t[:, cb - cb0, :],
                                    start=first, stop=last)
                                nc.tensor.matmul(
                                    phi[0:DH, j, :], mt[:, lsl, 128:D],
                                    st_t[:, cb - cb0, :],
                                    start=first, stop=last)
                        bs = slice(b0 * BIN, b1 * BIN)
                        nc.scalar.activation(
                            agg_lo[:, bs].rearrange("p (c x) -> p c x",
                                                    x=BIN),
                            plo[:, 0:npk, :], AF.Copy)
                        nc.scalar.activation(
                            agg_hi[0:DH, bs].rearrange("p (c x) -> p c x",
                                                       x=BIN),
                            phi[0:DH, 0:npk, :], AF.Copy)

                # ---------------- phase C: GRU -----------------------------
                nchunks = 0 if "gru" in skip else math.ceil(NSH / NCH)
                for chi in range(nchunks):
                    n0 = chi * NCH
                    n1 = min(n0 + NCH, NSH)
                    w = n1 - n0
                    nsl = slice(n0, n1)

                    def mm_into(pt, j, col0, part, wgt, rhs_lo, rhs_hi,
                                first, last):
                        mm = pt[0:part, j, 0:w]
                        cs = slice(col0, col0 + part)
                        nc.tensor.matmul(mm, wgt[:, 0, cs], rhs_lo[:, nsl],
                                         start=first, stop=False)
                        nc.tensor.matmul(mm, wgt[0:KH, 1, cs],
                                         rhs_hi[0:KH, nsl],
                                         start=False, stop=last)

                    # pass 1: r (slice 0) and z (slice 1), gi + gh accumulated
                    p1_lo = psC.tile([128, 2, NCH], dt.float32, tag="p1_lo")
                    p1_hi = psC.tile([128, 2, NCH], dt.float32, tag="p1_hi")
                    for j, col0 in ((0, 0), (1, D)):
                        mm_into(p1_lo, j, col0, 128, wih_s, agg_lo, agg_hi,
                                True, False)
                        mm_into(p1_lo, j, col0, 128, whh_s, h_lo, h_hi,
                                False, True)
                        mm_into(p1_hi, j, col0 + 128, DH, wih_s, agg_lo,
                                agg_hi, True, False)
                        mm_into(p1_hi, j, col0 + 128, DH, whh_s, h_lo, h_hi,
                                False, True)

                    rt_lo = sbC.tile([128, NCH], dt.float32, tag="rt_lo")
                    rt_hi = sbC.tile([128, NCH], dt.float32, tag="rt_hi")
                    zt_lo = sbC.tile([128, NCH], dt.float32, tag="zt_lo")
                    zt_hi = sbC.tile([128, NCH], dt.float32, tag="zt_hi")
                    nc.scalar.activation(rt_lo[:, 0:w], p1_lo[:, 0, 0:w],
                                         AF.Sigmoid)
                    nc.scalar.activation(rt_hi[0:DH, 0:w],
                                         p1_hi[0:DH, 0, 0:w], AF.Sigmoid)
                    nc.scalar.activation(zt_lo[:, 0:w], p1_lo[:, 1, 0:w],
                                         AF.Sigmoid)
                    nc.scalar.activation(zt_hi[0:DH, 0:w],
                                         p1_hi[0:DH, 1, 0:w], AF.Sigmoid)

                    # pass 2: i_n (slice 0) and h_n (slice 1)
                    p2_lo = psC.tile([128, 2, NCH], dt.float32, tag="p2_lo")
                    p2_hi = psC.tile([128, 2, NCH], dt.float32, tag="p2_hi")
                    mm_into(p2_lo, 0, 2 * D, 128, wih_s, agg_lo, agg_hi,
                            True, True)
                    mm_into(p2_hi, 0, 2 * D + 128, DH, wih_s, agg_lo, agg_hi,
                            True, True)
                    mm_into(p2_lo, 1, 2 * D, 128, whh_s, h_lo, h_hi,
                            True, True)
                    mm_into(p2_hi, 1, 2 * D + 128, DH, whh_s, h_lo, h_hi,
                            True, True)

                    t_lo = sbC.tile([128, NCH], dt.float32, tag="t_lo")
                    t_hi = sbC.tile([128, NCH], dt.float32, tag="t_hi")
                    n_lo = sbC.tile([128, NCH], dt.float32, tag="n_lo")
                    n_hi = sbC.tile([128, NCH], dt.float32, tag="n_hi")

                    for (rt, zt, p2, tt, nn, hh, part) in (
                        (rt_lo, zt_lo, p2_lo, t_lo, n_lo, h_lo, 128),
                        (rt_hi, zt_hi, p2_hi, t_hi, n_hi, h_hi, DH),
                    ):
                        ps = slice(0, part)
                        ws = slice(0, w)
                        # t = r * h_n ; s = i_n + t ; n = tanh(s)
                        nc.vector.tensor_mul(tt[ps, ws], rt[ps, ws],
                                             p2[ps, 1, ws])
                        nc.vector.tensor_add(tt[ps, ws], tt[ps, ws],
                                             p2[ps, 0, ws])
                        nc.scalar.activation(nn[ps, ws], tt[ps, ws], AF.Tanh)
                        # d = h - n ; e = z*d ; h' = n + e
                        nc.vector.tensor_sub(tt[ps, ws], hh[ps, nsl],
                                             nn[ps, ws])
                        nc.vector.tensor_mul(tt[ps, ws], tt[ps, ws],
                                             zt[ps, ws])
                        nc.vector.tensor_add(hh[ps, nsl], nn[ps, ws],
                                             tt[ps, ws])

            phase_stack.close()
            # ---------------- final relu + mask + max ----------------------
            with tc.tile_pool(name="fin", bufs=1) as fin:
                if "fin" in skip:
                    ytmp = fin.tile([128, 2], dt.float32, tag="ytmp")
                    nc.scalar.activation(ytmp[:], h_lo[:, 0:2], AF.Copy)
                    nc.sync.dma_start(y_out.ap(), ytmp[:])
                else:
                    y_s = fin.tile([128, 2], dt.float32, tag="y")
                    mask_s = fin.tile([128, NSH], dt.bfloat16, tag="mask")
                    nc.sync.dma_start(mask_s[:], mask_in.ap())
                    for col, hh in ((0, h_lo), (1, h_hi)):
                        rl = fin.tile([128, NSH], dt.bfloat16, tag="rl")
                        nc.scalar.activation(rl[:], hh[:], AF.Relu)
                        nc.vector.tensor_mul(rl[:], rl[:], mask_s[:])
                        nc.vector.reduce_max(y_s[:, col:col + 1], rl[:],
                                             axis=mybir.AxisListType.X)
                    nc.sync.dma_start(y_out.ap(), y_s[:])

    nc.compile()
    return nc


# ------------------------------------------------------------------ driver ----

def postprocess(cfg: Cfg, y_all, cls_w, cls_b):
    # y_all: [NC, 128, 2] f32; col0 = feats 0:128, col1 rows 0:72 = feats 128:200
    DH = cfg.D - 128
    pooled = np.zeros(cfg.D, np.float64)
    ym = np.max(np.stack(y_all), axis=0)         # [128, 2]
    pooled[0:128] = ym[:, 0]
    pooled[128:cfg.D] = ym[0:DH, 1]
    logits = pooled @ np.asarray(cls_w, np.float64).T + np.asarray(
        cls_b, np.float64)
    e = np.exp(logits - logits.max())
    sm = e / e.sum()
    return sm[None, :].astype(np.float32)


def kernel(**inputs):
    import sys
    if '/opt/trn_rl_repo' not in sys.path:
        sys.path.insert(0, '/opt/trn_rl_repo')
    from concourse import bass_utils
    cfg = Cfg()
    plan = build_plan(cfg, np.asarray(inputs["edge_index"]))
    in_maps = host_inputs(cfg, plan, inputs)
    nc = build_program(cfg, plan)
    res = bass_utils.run_bass_kernel_spmd(nc, in_maps,
                                          core_ids=list(range(cfg.NC)))
    y_all = [res.results[c]["y"] for c in range(cfg.NC)]
    return postprocess(cfg, y_all, inputs["cls_w"], inputs["cls_b"])

